# revision 24
# baseline (speedup 1.0000x reference)
"""Multi-head attention + layernorm Bass kernel for Trainium2, 8 cores.

Problem: B=8, S=1024, D=768, H=12 heads x DH=64, key-padding mask, softmax,
output projection, layernorm.  Sharding: pure data parallelism - one batch
element per NeuronCore, no collectives.

Design (vs the 264us baseline):
  - The kernel core is ACT(exp)-bound: 96 exp tiles of [128,1024] ~= 110us.
    One globally software-pipelined stream runs all 96 (pair, query-half,
    key-chunk) steps: scores(step s) and ctx(step s-1) are emitted together
    so the in-order PE queue never stalls on a just-issued exp, and the exp
    queue never drains at tile boundaries.
  - All later pairs' q/k/v projection matmuls are interleaved as background
    PE work inside the stream (the PE has ~40% slack while ACT works), with
    hard deadlines so program order always matches dependency order.
  - Softmax denominators ride the v-ones-column trick (row 64 of the ctx
    psum), reciprocal via reciprocal_approx_fast (must run at partition base
    0), broadcast per query via one K=65 f16 selector matmul.
  - Non-critical weight DMAs are semaphore-deferred behind the first exps so
    the startup DMA bandwidth all goes to xt + pair-0/quad-0 weights.
  - Out-projection blocks 0-3 (query half 0) are emitted right after pair 5's
    first query half completes, overlapping pair 5's second half; the rest
    pipelines 3-deep through a dedicated psum tag.
"""

import numpy as np

B, S, D, H, DH = 8, 1024, 768, 12, 64
NPAIR, NQUAD = H // 2, H // 4
SBLK = S // 128      # 8 key/row chunks
DCH = D // 128       # 6 contraction chunks
LN_EPS = 1e-5
NEG_MASK = -30.0

_PROGRAMS = {}


def _build_program(trivial_ln):
    import concourse.bass as bass
    from concourse import bacc
    import concourse.tile as tile
    from concourse.tile import add_dep_helper
    import concourse.mybir as mybir
    from contextlib import ExitStack

    F32 = mybir.dt.float32
    F32R = mybir.dt.float32r
    F16 = mybir.dt.float16
    AF = mybir.ActivationFunctionType

    nc = bacc.Bacc("TRN2", target_bir_lowering=False)

    xt_d = nc.dram_tensor("xt", [128, DCH * S], F16, kind="ExternalInput")
    wq_d = nc.dram_tensor("wq", [NPAIR, 128, DCH * 128], F16, kind="ExternalInput")
    wk_d = nc.dram_tensor("wk", [NPAIR, 128, DCH * 128], F16, kind="ExternalInput")
    wv_d = nc.dram_tensor("wv", [NQUAD, 128, DCH * 260], F16, kind="ExternalInput")
    wo_d = nc.dram_tensor("wo", [128, DCH * D], F16, kind="ExternalInput")
    bqk_d = nc.dram_tensor("bqk", [128, 2 * NPAIR], F32, kind="ExternalInput")
    bv_d = nc.dram_tensor("bv", [1, NQUAD * 260], F32, kind="ExternalInput")
    maskb_d = nc.dram_tensor("maskb", [128, SBLK], F32, kind="ExternalInput")
    gamma_d = nc.dram_tensor("gamma", [1, D], F32, kind="ExternalInput")
    beta_d = nc.dram_tensor("beta", [1, D], F32, kind="ExternalInput")
    sel_d = nc.dram_tensor("sel", [65, 128], F16, kind="ExternalInput")
    onesr_d = nc.dram_tensor("onesr", [1, 128], F32R, kind="ExternalInput")
    bor_d = nc.dram_tensor("bor", [1, D], F32R, kind="ExternalInput")
    out_d = nc.dram_tensor("out", [S, D], F32, kind="ExternalOutput")

    with tile.TileContext(nc) as tc, ExitStack() as ctx:
        const = ctx.enter_context(tc.tile_pool(name="const", bufs=1))
        xt_p = ctx.enter_context(tc.tile_pool(name="xt_p", bufs=1))
        w_p = ctx.enter_context(tc.tile_pool(name="w_p", bufs=1))
        qk_p = ctx.enter_context(tc.tile_pool(name="qk_p", bufs=1))
        v_p = ctx.enter_context(tc.tile_pool(name="v_p", bufs=1))
        e_p = ctx.enter_context(tc.tile_pool(name="e_p", bufs=1))
        cx_p = ctx.enter_context(tc.tile_pool(name="cx_p", bufs=1))
        z_p = ctx.enter_context(tc.tile_pool(name="z_p", bufs=1))
        # PSUM (8 banks): scores 2x[128,1024]=4, proj 2x[128,512]=2,
        # ctx 2x[65,512]=2.  The out-projection's "pso" tag (3x2 banks)
        # starts allocating as the scores/proj/cx slots retire.
        ps = ctx.enter_context(tc.tile_pool(name="ps", bufs=1, space="PSUM"))

        # ---- critical input DMAs: xt chunks + pair0/quad0 weights ----
        xta0 = xt_p.tile([128, DCH // 2, S], F16, name="xta0")
        xta1 = xt_p.tile([128, DCH // 2, S], F16, name="xta1")
        for eng, tile_, base in ((nc.sync, xta0, 0), (nc.scalar, xta1, DCH // 2)):
            for cc in range(DCH // 2):
                eng.dma_start(out=tile_[:, cc, :],
                              in_=xt_d[:, (base + cc) * S:(base + cc + 1) * S])
        xt = [xta0[:, c, :] for c in range(DCH // 2)] + \
             [xta1[:, c, :] for c in range(DCH // 2)]

        bqk_t = const.tile([128, 2 * NPAIR], F32)
        nc.sync.dma_start(out=bqk_t, in_=bqk_d[:, :])
        bv_t = const.tile([128, NQUAD * 260], F32)
        nc.sync.dma_start(out=bv_t, in_=bv_d[0:1, :].to_broadcast([128, NQUAD * 260]))
        mask_t = const.tile([128, SBLK], F32)
        nc.sync.dma_start(out=mask_t, in_=maskb_d[:, :])
        sel_t = const.tile([65, 128], F16)
        nc.sync.dma_start(out=sel_t, in_=sel_d[:, :])
        onesr_t = const.tile([1, 128], F32R)
        nc.sync.dma_start(out=onesr_t, in_=onesr_d[:, :])
        bor_t = const.tile([1, D], F32R)
        nc.sync.dma_start(out=bor_t, in_=bor_d[:, :])
        eps_t = const.tile([128, 1], F32)
        nc.vector.memset(eps_t, LN_EPS)

        wv_ts, wq_ts, wk_ts = [None] * NQUAD, [None] * NPAIR, [None] * NPAIR
        dma_insts = {}

        def load_wv(q, eng):
            wvq = w_p.tile([128, DCH, 260], F16, name="wvq", bufs=NQUAD)
            dma_insts["wv%d" % q] = eng.dma_start(out=wvq, in_=wv_d[q])
            wv_ts[q] = [wvq[:, c, :] for c in range(DCH)]

        def load_wqk(p, eng):
            wqp = w_p.tile([128, DCH, 128], F16, name="wqp", bufs=NPAIR)
            dma_insts["wq%d" % p] = eng.dma_start(out=wqp, in_=wq_d[p])
            wq_ts[p] = [wqp[:, c, :] for c in range(DCH)]
            wkp = w_p.tile([128, DCH, 128], F16, name="wkp", bufs=NPAIR)
            dma_insts["wk%d" % p] = eng.dma_start(out=wkp, in_=wk_d[p])
            wk_ts[p] = [wkp[:, c, :] for c in range(DCH)]

        load_wqk(0, nc.gpsimd)
        load_wv(0, nc.gpsimd)
        # non-critical weights: issued on the gpsimd queue but semaphore-
        # deferred behind early exps so they don't steal startup bandwidth
        load_wqk(1, nc.gpsimd)
        load_wv(1, nc.gpsimd)
        load_wqk(2, nc.gpsimd)
        load_wv(2, nc.gpsimd)
        load_wqk(3, nc.gpsimd)
        load_wqk(4, nc.gpsimd)
        load_wqk(5, nc.gpsimd)
        woa = w_p.tile([128, DCH, D], F16, name="woa", bufs=1)
        dma_insts["wo"] = nc.gpsimd.dma_start(out=woa, in_=wo_d[:, :])
        wo_t = [woa[:, c, :] for c in range(DCH)]
        # gate: key -> exp step index after which the DMA may start
        dma_gate = {"wq1": 2, "wk1": 2, "wv1": 3, "wq2": 8, "wk2": 8,
                    "wv2": 9, "wq3": 10, "wk3": 10, "wq4": 16, "wk4": 16,
                    "wq5": 17, "wk5": 17, "wo": 18}
        exp_insts = []

        # ---- projection chunk emitters ----
        qt_tiles, kt_tiles = {}, {}
        v_sb = {}

        def qk_chunk(p, which, half):
            w_ts = wq_ts[p] if which == "q" else wk_ts[p]
            tiles = qt_tiles if which == "q" else kt_tiles
            if p not in tiles:
                tiles[p] = qk_p.tile([128, S], F16, name=which + "t_sb", bufs=3)
            dst = tiles[p]
            psq = ps.tile([128, 512], F32, name="psqk", tag="proj", bufs=2)
            mms = []
            for c in range(DCH):
                mms.append(lambda c=c, psq=psq, w_ts=w_ts, half=half:
                           nc.tensor.matmul(
                               psq, w_ts[c], xt[c][:, half * 512:(half + 1) * 512],
                               start=(c == 0), stop=(c == DCH - 1)))
            boff = p if which == "q" else NPAIR + p

            def evac(psq=psq, dst=dst, half=half, boff=boff):
                with tc.high_priority(offset=400):
                    nc.vector.tensor_scalar_add(
                        out=dst[:, half * 512:(half + 1) * 512], in0=psq,
                        scalar1=bqk_t[:, boff:boff + 1])
            return mms, evac

        def v_chunk(q, s):
            psv = ps.tile([128, 260], F32, name="psv", tag="proj", bufs=2)
            mms = []
            for c in range(DCH):
                mms.append(lambda c=c, psv=psv, q=q, s=s: nc.tensor.matmul(
                    psv, xt[c][:, s * 128:(s + 1) * 128], wv_ts[q][c],
                    start=(c == 0), stop=(c == DCH - 1)))
            vt = v_p.tile([128, 260], F16, name="v_sb", bufs=3 * SBLK)
            v_sb[(q, s)] = vt

            def evac(psv=psv, vt=vt, q=q):
                with tc.high_priority(offset=400):
                    nc.vector.tensor_add(out=vt, in0=psv,
                                         in1=bv_t[:, q * 260:(q + 1) * 260])
            return mms, evac

        # ---- background stream with hard deadlines ----
        bg = []

        def push_qk(p):
            for which, half in (("q", 0), ("k", 0), ("q", 1), ("k", 1)):
                bg.append(qk_chunk(p, which, half))

        def push_v(q):
            for s in range(SBLK):
                bg.append(v_chunk(q, s))

        # chunks ordered by deadline (step index by which the chunk's
        # output must exist); q/k halves of pair p are due just before the
        # pair's query halves start, v-quad n chunk s just before ctx(2n,0,s)
        bg_deadlines = []

        def push_qk(p):
            for (which, half), dl in ((("q", 0), 16 * p - 4),
                                      (("k", 0), 16 * p - 2),
                                      (("q", 1), 16 * p + 4),
                                      (("k", 1), 16 * p + 6)):
                bg.append(qk_chunk(p, which, half))
                bg_deadlines.append(dl)

        def push_v(q):
            for s in range(SBLK):
                bg.append(v_chunk(q, s))
                bg_deadlines.append(32 * q + s - 1)

        push_qk(1)
        push_qk(2)
        push_v(1)
        push_qk(3)
        push_qk(4)
        push_v(2)
        push_qk(5)
        order = sorted(range(len(bg)), key=lambda i: bg_deadlines[i])
        bg[:] = [bg[i] for i in order]
        bg_deadlines[:] = sorted(bg_deadlines)
        # cumulative end position of each chunk in matmuls
        bg_pos = []
        acc = 0
        for mms, _ in bg:
            acc += len(mms)
            bg_pos.append(acc)

        bg_state = {"mms": [], "evac": None, "emitted": 0}

        def bg_pop(n):
            for _ in range(n):
                if not bg_state["mms"]:
                    if bg_state["evac"] is not None:
                        bg_state["evac"]()
                        bg_state["evac"] = None
                    if not bg:
                        return
                    bg_state["mms"], bg_state["evac"] = bg.pop(0)
                bg_state["mms"].pop(0)()
                bg_state["emitted"] += 1
            if not bg_state["mms"] and bg_state["evac"] is not None:
                bg_state["evac"]()
                bg_state["evac"] = None

        def bg_flush_to(pos):
            while bg_state["emitted"] < pos and (
                    bg or bg_state["mms"] or bg_state["evac"]):
                bg_pop(6)

        def bg_flush():
            while bg or bg_state["mms"] or bg_state["evac"]:
                bg_pop(6)

        # ---- startup: pair-0 q/k dense; quad-0 v rides inline in pair 0 ----
        for which, half in (("q", 0), ("k", 0), ("q", 1), ("k", 1)):
            mms, evac = qk_chunk(0, which, half)
            for m in mms:
                m()
            evac()
        startup_v = [v_chunk(0, s) for s in range(SBLK)]

        # denominator scratch: rows 1-63 stay 1.0 forever (reciprocal_approx
        # must run at partition base 0; garbage rows would turn into NaN via
        # 0*inf in the selector matmul)
        rpk_bufs = [z_p.tile([65, 512], F32, name="rpk%d" % i, bufs=1)
                    for i in range(3)]
        rinv_bufs = [z_p.tile([65, 512], F32, name="rinv%d" % i, bufs=1)
                     for i in range(3)]
        rinv16_bufs = [z_p.tile([65, 512], F16, name="rinv16%d" % i, bufs=1)
                       for i in range(3)]
        for t in rpk_bufs:
            nc.vector.memset(t, 1.0)

        ctxt = [cx_p.tile([128, S], F16, name="ctxt", bufs=NPAIR)
                for _ in range(NPAIR)]

        # ---- the fused attention stream ----
        steps = [(p, iblk, j) for p in range(NPAIR) for iblk in range(2)
                 for j in range(SBLK)]
        ets = {}
        pcx_cur = {}

        def emit_scores(p, iblk, j):
            qt, kt = qt_tiles[p], kt_tiles[p]
            pst = ps.tile([128, 1024], F32, name="psst", tag="st", bufs=2)
            nc.tensor.matmul(
                pst[:, 0:512], kt[0:64, j * 128:(j + 1) * 128],
                qt[0:64, iblk * 512:(iblk + 1) * 512],
                start=True, stop=True, tile_position=(0, 0))
            nc.tensor.matmul(
                pst[:, 512:1024], kt[64:128, j * 128:(j + 1) * 128],
                qt[64:128, iblk * 512:(iblk + 1) * 512],
                start=True, stop=True, tile_position=(64, 0))
            et = e_p.tile([128, 1024], F16, name="expt", bufs=4)
            ei = nc.scalar.activation(et, pst, AF.Exp, bias=mask_t[:, j:j + 1])
            exp_insts.append(ei)
            ets[(p, iblk, j)] = et

        def emit_ctx(p, iblk, j):
            q, l0 = divmod(2 * p, 4)
            if j == 0:
                pcx_cur[(p, iblk)] = [
                    ps.tile([65, 512], F32, name="pscx", tag="cx", bufs=2)
                    for _ in range(2)]
            pcx = pcx_cur[(p, iblk)]
            etc = ets.pop((p, iblk, j))
            for idx in range(2):
                vsl = v_sb[(q, j)][:, (l0 + idx) * 65:(l0 + idx + 1) * 65]
                nc.tensor.matmul(pcx[idx], vsl,
                                 etc[:, idx * 512:(idx + 1) * 512],
                                 start=(j == 0), stop=(j == SBLK - 1))

        def emit_finish(p, iblk):
            # evacuate ctx + denominators, then normalize off-path
            ct = ctxt[p]
            pcx = pcx_cur.pop((p, iblk))
            rpk = rpk_bufs[(2 * p + iblk) % 3]
            with tc.high_priority(offset=500):
                for idx in range(2):
                    nc.vector.tensor_copy(
                        out=ct[idx * 64:(idx + 1) * 64,
                               iblk * 512:(iblk + 1) * 512],
                        in_=pcx[idx][0:64, :])
                    nc.vector.tensor_copy(out=rpk[idx * 64:idx * 64 + 1, :],
                                          in_=pcx[idx][64:65, :])
            rinv = rinv_bufs[(2 * p + iblk) % 3]
            rinv16 = rinv16_bufs[(2 * p + iblk) % 3]
            with tc.high_priority(offset=-300):
                with nc.allow_low_precision(reason="softmax denom approx"):
                    nc.vector.reciprocal_approx_fast(out=rinv, in_=rpk)
                nc.vector.tensor_copy(out=rinv16, in_=rinv)
                pbc = ps.tile([128, 512], F32, name="psbc", tag="proj", bufs=2)
                nc.tensor.matmul(pbc, sel_t, rinv16[:, :], start=True,
                                 stop=True)
                for idx in range(2):
                    csl = ct[idx * 64:(idx + 1) * 64,
                             iblk * 512:(iblk + 1) * 512]
                    nc.vector.tensor_mul(out=csl, in0=csl,
                                         in1=pbc[idx * 64:(idx + 1) * 64, :])

        if not trivial_ln:
            gamma_t = const.tile([128, D], F32)
            nc.sync.dma_start(out=gamma_t,
                              in_=gamma_d[0:1, :].to_broadcast([128, D]))
            beta_t = const.tile([128, D], F32)
            nc.sync.dma_start(out=beta_t,
                              in_=beta_d[0:1, :].to_broadcast([128, D]))

        outproj_fin = []

        def emit_outproj(s, chunked=False):
            # chunked: run through the (free) proj-tag slots so the block can
            # overlap pair 5's second query half, whose st slots are busy
            if chunked:
                pso_a = ps.tile([128, 512], F32, name="pso_a", tag="proj",
                                bufs=2)
                pso_b = ps.tile([128, 256], F32, name="pso_b", tag="proj",
                                bufs=2)
                chunks = ((pso_a, 0, 512), (pso_b, 512, 768))
            else:
                pso = ps.tile([128, D], F32, name="pso", tag="st", bufs=2,
                              padded_shape=[128, 1024])
                chunks = ((pso, 0, 512), (pso, 512, 768))
            for buf, d0, d1 in chunks:
                dst = buf if chunked else buf[:, d0:d1]
                if chunked and d0 == 512:
                    dst = buf
                for p in range(NPAIR):
                    nc.tensor.matmul(
                        dst,
                        ctxt[p][:, s * 128:(s + 1) * 128],
                        wo_t[p][:, d0:d1],
                        start=(p == 0), stop=False)
                nc.tensor.matmul(dst, onesr_t, bor_t[:, d0:d1],
                                 start=False, stop=True)
            stats = z_p.tile([128, 3, 6], F32, name="stats", bufs=4)
            if chunked:
                srcs = (pso_a[:, 0:256], pso_a[:, 256:512], pso_b[:, :])
            else:
                srcs = tuple(pso[:, g * 256:(g + 1) * 256] for g in range(3))
            for g in range(3):
                nc.vector.bn_stats(out=stats[:, g, :], in_=srcs[g])
            mv = z_p.tile([128, 2], F32, name="mv", bufs=4)
            nc.vector.bn_aggr(out=mv, in_=stats)

            def fin(s=s, mv=mv, chunks=chunks, chunked=chunked):
                # ACT Sqrt deferred past the last exp so the table set is
                # switched exactly once
                stdv = z_p.tile([128, 1], F32, name="stdv", bufs=3)
                nc.scalar.activation(stdv, mv[:, 1:2], AF.Sqrt, bias=eps_t)
                rstd = z_p.tile([128, 1], F32, name="rstd", bufs=3)
                nc.vector.reciprocal(out=rstd, in_=stdv)
                nmr = z_p.tile([128, 1], F32, name="nmr", bufs=3)
                nc.vector.tensor_scalar(out=nmr, in0=mv[:, 0:1], scalar1=rstd,
                                        scalar2=-1.0,
                                        op0=mybir.AluOpType.mult,
                                        op1=mybir.AluOpType.mult)
                z = z_p.tile([128, D], F32, name="z_sb", bufs=3)
                if chunked:
                    nc.scalar.activation(z[:, 0:512], chunks[0][0],
                                         AF.Identity, bias=nmr, scale=rstd)
                    nc.scalar.activation(z[:, 512:768], chunks[1][0],
                                         AF.Identity, bias=nmr, scale=rstd)
                else:
                    nc.scalar.activation(z, chunks[0][0], AF.Identity,
                                         bias=nmr, scale=rstd)
                if not trivial_ln:
                    nc.vector.tensor_mul(out=z, in0=z, in1=gamma_t)
                    nc.vector.tensor_add(out=z, in0=z, in1=beta_t)
                nc.sync.dma_start(out=out_d[s * 128:(s + 1) * 128, :], in_=z)
            outproj_fin.append(fin)

        bg_done0 = len(bg)

        def bg_required(sti):
            # matmuls that must be emitted by now to satisfy every upcoming
            # chunk deadline at a constant rate
            emitted = bg_state["emitted"]
            req = 0
            base = bg_done0 - len(bg)
            first = base - 1 if (bg_state["mms"] or bg_state["evac"]) else base
            for ci in range(max(0, first), bg_done0):
                pos, dl = bg_pos[ci], bg_deadlines[ci]
                if pos <= emitted:
                    continue
                if dl <= sti:
                    req = max(req, pos - emitted)
                else:
                    req = max(req, -(-(pos - emitted) // (dl - sti)))
            return req

        prev = None
        for sti in range(len(steps) + 1):
            cur = steps[sti] if sti < len(steps) else None
            if cur is not None:
                p, iblk, j = cur
                emit_scores(p, iblk, j)
                if p == 0 and iblk == 0:
                    mms, evac = startup_v[j]
                    for m in mms:
                        m()
                    evac()
            # background projection matmuls keep the PE dense while ACT
            # works through the exp queue; pace to the chunk deadlines
            bg_pop(max(2, bg_required(sti)))
            if prev is not None:
                pp, piblk, pj = prev
                emit_ctx(pp, piblk, pj)
                if pj == SBLK - 1:
                    emit_finish(pp, piblk)
                    if pp == NPAIR - 1 and piblk == 0:
                        # query half 0 fully normalized: overlap the first
                        # out-projection blocks with pair 5's half 1
                        for s in range(3):
                            emit_outproj(s, chunked=True)
            prev = cur

        bg_flush()
        for s in range(3, SBLK):
            emit_outproj(s)
        for fin in outproj_fin:
            fin()

        # defer non-critical weight DMAs behind early exps
        for key, gate in dma_gate.items():
            if gate < len(exp_insts):
                add_dep_helper(dma_insts[key].ins, exp_insts[gate].ins,
                               sync=True,
                               reason="defer weight DMA past startup")

    nc.compile()
    return nc


def _host_inputs(inputs):
    x = np.asarray(inputs["input_tensor"], np.float32)
    mask = np.asarray(inputs["attention_mask"])
    Wq = np.asarray(inputs["Wq"], np.float32)
    bq = np.asarray(inputs["bq"], np.float32)
    Wk = np.asarray(inputs["Wk"], np.float32)
    bk = np.asarray(inputs["bk"], np.float32)
    Wv = np.asarray(inputs["Wv"], np.float32)
    bv = np.asarray(inputs["bv"], np.float32)
    Wo = np.asarray(inputs["Wo"], np.float32)
    bo = np.asarray(inputs["bo"], np.float32)
    gamma = np.asarray(inputs["gamma"], np.float32)
    beta = np.asarray(inputs["beta"], np.float32)

    scale = np.float32(1.0 / np.sqrt(DH))
    wq_flat = np.ascontiguousarray(
        (Wq * scale).transpose(1, 0, 2).reshape(D, D))
    wk_flat = np.ascontiguousarray(Wk.transpose(1, 0, 2).reshape(D, D))
    bq_s = (bq * scale).reshape(D)
    bk_s = bk.reshape(D)

    wv_aug = np.zeros((D, NQUAD * 260), np.float32)
    bv_aug = np.zeros((1, NQUAD * 260), np.float32)
    for h in range(H):
        q, l = divmod(h, 4)
        base = q * 260 + l * 65
        wv_aug[:, base:base + 64] = Wv[h]
        bv_aug[0, base:base + 64] = bv[h]
        bv_aug[0, base + 64] = 1.0

    bqk = np.zeros((128, 2 * NPAIR), np.float32)
    for p in range(NPAIR):
        bqk[:, p] = bq_s[p * 128:(p + 1) * 128]
        bqk[:, NPAIR + p] = bk_s[p * 128:(p + 1) * 128]

    def sbuf_layout(w, width):
        n = w.shape[1] // width
        return np.ascontiguousarray(
            w.reshape(DCH, 128, n, width).transpose(2, 1, 0, 3).reshape(
                n, 128, DCH * width).astype(np.float16))

    shared = {
        "wq": sbuf_layout(wq_flat, 128), "wk": sbuf_layout(wk_flat, 128),
        "wv": sbuf_layout(wv_aug, 260),
        "wo": sbuf_layout(np.ascontiguousarray(Wo), D)[0],
        "bqk": bqk, "bv": bv_aug,
        "gamma": gamma.reshape(1, D), "beta": beta.reshape(1, D),
        "sel": _sel_matrix(),
        "onesr": np.ones((1, 128), np.float32),
        "bor": bo.reshape(1, D).copy(),
    }
    in_maps = []
    for b in range(B):
        mb = np.where(mask[b], 0.0, NEG_MASK).astype(np.float32)
        in_maps.append({
            **shared,
            "xt": np.ascontiguousarray(
                x[b].T.reshape(DCH, 128, S).transpose(1, 0, 2).reshape(
                    128, DCH * S).astype(np.float16)),
            "maskb": np.ascontiguousarray(mb.reshape(SBLK, 128).T),
        })
    return in_maps


def _sel_matrix():
    sel = np.zeros((65, 128), np.float16)
    sel[0, 0:64] = 1.0
    sel[64, 64:128] = 1.0
    return sel


def _trivial_ln(inputs):
    gamma = np.asarray(inputs["gamma"], np.float32)
    beta = np.asarray(inputs["beta"], np.float32)
    return bool(np.all(gamma == 1.0) and np.all(beta == 0.0))


def _get_program(trivial=True):
    if trivial not in _PROGRAMS:
        _PROGRAMS[trivial] = _build_program(trivial)
    return _PROGRAMS[trivial]


def kernel(**inputs):
    from concourse.bass_utils import run_bass_kernel_spmd

    nc = _get_program(_trivial_ln(inputs))
    in_maps = _host_inputs(inputs)
    res = run_bass_kernel_spmd(nc, in_maps, list(range(B)))
    return np.stack([res.results[b]["out"] for b in range(B)], axis=0)


if __name__ == "__main__":
    rng = np.random.default_rng(0)
    demo = {
        "input_tensor": rng.standard_normal((B, S, D)).astype(np.float32),
        "attention_mask": np.ones((B, S), bool),
        "Wq": rng.standard_normal((H, D, DH)).astype(np.float32) * 0.03,
        "bq": rng.standard_normal((H, DH)).astype(np.float32) * 0.03,
        "Wk": rng.standard_normal((H, D, DH)).astype(np.float32) * 0.03,
        "bk": rng.standard_normal((H, DH)).astype(np.float32) * 0.03,
        "Wv": rng.standard_normal((H, D, DH)).astype(np.float32) * 0.03,
        "bv": rng.standard_normal((H, DH)).astype(np.float32) * 0.03,
        "Wo": rng.standard_normal((D, D)).astype(np.float32) * 0.03,
        "bo": rng.standard_normal((D,)).astype(np.float32) * 0.03,
        "gamma": np.ones((D,), np.float32),
        "beta": np.zeros((D,), np.float32),
    }
    out = kernel(**demo)
    print("kernel ran, out shape", out.shape, "finite:", np.isfinite(out).all())


# revision 25
# speedup vs baseline: 1.0415x; 1.0415x over previous
"""Multi-head attention + layernorm Bass kernel for Trainium2, 8 cores.

Problem: B=8, S=1024, D=768, H=12 heads x DH=64, key-padding mask, softmax,
output projection, layernorm.  Sharding: pure data parallelism - one batch
element per NeuronCore, no collectives.

Design (vs the 264us baseline):
  - The kernel core is ACT(exp)-bound: 96 exp tiles of [128,1024] ~= 110us.
    One globally software-pipelined stream runs all 96 (pair, query-half,
    key-chunk) steps: scores(step s) and ctx(step s-1) are emitted together
    so the in-order PE queue never stalls on a just-issued exp, and the exp
    queue never drains at tile boundaries.
  - All later pairs' q/k/v projection matmuls are interleaved as background
    PE work inside the stream (the PE has ~40% slack while ACT works), with
    hard deadlines so program order always matches dependency order.
  - Softmax denominators ride the v-ones-column trick (row 64 of the ctx
    psum), reciprocal via reciprocal_approx_fast (must run at partition base
    0), broadcast per query via one K=65 f16 selector matmul.
  - Non-critical weight DMAs are semaphore-deferred behind the first exps so
    the startup DMA bandwidth all goes to xt + pair-0/quad-0 weights.
  - Out-projection blocks 0-3 (query half 0) are emitted right after pair 5's
    first query half completes, overlapping pair 5's second half; the rest
    pipelines 3-deep through a dedicated psum tag.
"""

import numpy as np

B, S, D, H, DH = 8, 1024, 768, 12, 64
NPAIR, NQUAD = H // 2, H // 4
SBLK = S // 128      # 8 key/row chunks
DCH = D // 128       # 6 contraction chunks
LN_EPS = 1e-5
NEG_MASK = -30.0

_PROGRAMS = {}


def _build_program(trivial_ln):
    import concourse.bass as bass
    from concourse import bacc
    import concourse.tile as tile
    from concourse.tile import add_dep_helper
    import concourse.mybir as mybir
    from contextlib import ExitStack

    F32 = mybir.dt.float32
    F32R = mybir.dt.float32r
    F16 = mybir.dt.float16
    AF = mybir.ActivationFunctionType

    nc = bacc.Bacc("TRN2", target_bir_lowering=False)

    xt_d = nc.dram_tensor("xt", [128, DCH * S], F16, kind="ExternalInput")
    wq_d = nc.dram_tensor("wq", [NPAIR, 128, DCH * 128], F16, kind="ExternalInput")
    wk_d = nc.dram_tensor("wk", [NPAIR, 128, DCH * 128], F16, kind="ExternalInput")
    wv_d = nc.dram_tensor("wv", [NQUAD, 128, DCH * 260], F16, kind="ExternalInput")
    wo_d = nc.dram_tensor("wo", [128, DCH * D], F16, kind="ExternalInput")
    bqk_d = nc.dram_tensor("bqk", [128, 2 * NPAIR], F32, kind="ExternalInput")
    bv_d = nc.dram_tensor("bv", [1, NQUAD * 260], F32, kind="ExternalInput")
    maskb_d = nc.dram_tensor("maskb", [128, SBLK], F32, kind="ExternalInput")
    gamma_d = nc.dram_tensor("gamma", [1, D], F32, kind="ExternalInput")
    beta_d = nc.dram_tensor("beta", [1, D], F32, kind="ExternalInput")
    sel_d = nc.dram_tensor("sel", [65, 128], F16, kind="ExternalInput")
    onesr_d = nc.dram_tensor("onesr", [1, 128], F32R, kind="ExternalInput")
    bor_d = nc.dram_tensor("bor", [1, D], F32R, kind="ExternalInput")
    out_d = nc.dram_tensor("out", [S, D], F32, kind="ExternalOutput")

    with tile.TileContext(nc) as tc, ExitStack() as ctx:
        const = ctx.enter_context(tc.tile_pool(name="const", bufs=1))
        xt_p = ctx.enter_context(tc.tile_pool(name="xt_p", bufs=1))
        w_p = ctx.enter_context(tc.tile_pool(name="w_p", bufs=1))
        qk_p = ctx.enter_context(tc.tile_pool(name="qk_p", bufs=1))
        v_p = ctx.enter_context(tc.tile_pool(name="v_p", bufs=1))
        e_p = ctx.enter_context(tc.tile_pool(name="e_p", bufs=1))
        cx_p = ctx.enter_context(tc.tile_pool(name="cx_p", bufs=1))
        z_p = ctx.enter_context(tc.tile_pool(name="z_p", bufs=1))
        # PSUM (8 banks): scores 2x[128,1024]=4, proj 2x[128,512]=2,
        # ctx 2x[65,512]=2.  The out-projection's "pso" tag (3x2 banks)
        # starts allocating as the scores/proj/cx slots retire.
        ps = ctx.enter_context(tc.tile_pool(name="ps", bufs=1, space="PSUM"))

        # ---- critical input DMAs: xt chunks + pair0/quad0 weights ----
        xta0 = xt_p.tile([128, DCH // 2, S], F16, name="xta0")
        xta1 = xt_p.tile([128, DCH // 2, S], F16, name="xta1")
        for eng, tile_, base in ((nc.sync, xta0, 0), (nc.scalar, xta1, DCH // 2)):
            for cc in range(DCH // 2):
                eng.dma_start(out=tile_[:, cc, :],
                              in_=xt_d[:, (base + cc) * S:(base + cc + 1) * S])
        xt = [xta0[:, c, :] for c in range(DCH // 2)] + \
             [xta1[:, c, :] for c in range(DCH // 2)]

        bqk_t = const.tile([128, 2 * NPAIR], F32)
        nc.sync.dma_start(out=bqk_t, in_=bqk_d[:, :])
        bv_t = const.tile([128, NQUAD * 260], F32)
        nc.sync.dma_start(out=bv_t, in_=bv_d[0:1, :].to_broadcast([128, NQUAD * 260]))
        mask_t = const.tile([128, SBLK], F32)
        nc.sync.dma_start(out=mask_t, in_=maskb_d[:, :])
        sel_t = const.tile([65, 128], F16)
        nc.sync.dma_start(out=sel_t, in_=sel_d[:, :])
        onesr_t = const.tile([1, 128], F32R)
        nc.sync.dma_start(out=onesr_t, in_=onesr_d[:, :])
        bor_t = const.tile([1, D], F32R)
        nc.sync.dma_start(out=bor_t, in_=bor_d[:, :])
        eps_t = const.tile([128, 1], F32)
        nc.vector.memset(eps_t, LN_EPS)

        wv_ts, wq_ts, wk_ts = [None] * NQUAD, [None] * NPAIR, [None] * NPAIR
        dma_insts = {}

        def load_wv(q, eng):
            wvq = w_p.tile([128, DCH, 260], F16, name="wvq", bufs=NQUAD)
            dma_insts["wv%d" % q] = eng.dma_start(out=wvq, in_=wv_d[q])
            wv_ts[q] = [wvq[:, c, :] for c in range(DCH)]

        def load_wqk(p, eng):
            wqp = w_p.tile([128, DCH, 128], F16, name="wqp", bufs=NPAIR)
            dma_insts["wq%d" % p] = eng.dma_start(out=wqp, in_=wq_d[p])
            wq_ts[p] = [wqp[:, c, :] for c in range(DCH)]
            wkp = w_p.tile([128, DCH, 128], F16, name="wkp", bufs=NPAIR)
            dma_insts["wk%d" % p] = eng.dma_start(out=wkp, in_=wk_d[p])
            wk_ts[p] = [wkp[:, c, :] for c in range(DCH)]

        load_wqk(0, nc.gpsimd)
        load_wv(0, nc.gpsimd)
        # non-critical weights: issued on the gpsimd queue but semaphore-
        # deferred behind early exps so they don't steal startup bandwidth
        load_wqk(1, nc.gpsimd)
        load_wv(1, nc.gpsimd)
        load_wqk(2, nc.gpsimd)
        load_wv(2, nc.gpsimd)
        load_wqk(3, nc.gpsimd)
        load_wqk(4, nc.gpsimd)
        load_wqk(5, nc.gpsimd)
        woa = w_p.tile([128, DCH, D], F16, name="woa", bufs=1)
        dma_insts["wo"] = nc.gpsimd.dma_start(out=woa, in_=wo_d[:, :])
        wo_t = [woa[:, c, :] for c in range(DCH)]
        # gate: key -> exp step index after which the DMA may start
        dma_gate = {"wq1": 2, "wk1": 2, "wv1": 3, "wq2": 8, "wk2": 8,
                    "wv2": 9, "wq3": 10, "wk3": 10, "wq4": 16, "wk4": 16,
                    "wq5": 17, "wk5": 17, "wo": 18}
        exp_insts = []

        # ---- projection chunk emitters ----
        qt_tiles, kt_tiles = {}, {}
        v_sb = {}

        def qk_chunk(p, which, half):
            w_ts = wq_ts[p] if which == "q" else wk_ts[p]
            tiles = qt_tiles if which == "q" else kt_tiles
            if p not in tiles:
                tiles[p] = qk_p.tile([128, S], F16, name=which + "t_sb", bufs=3)
            dst = tiles[p]
            psq = ps.tile([128, 512], F32, name="psqk", tag="proj", bufs=2)
            mms = []
            for c in range(DCH):
                mms.append(lambda c=c, psq=psq, w_ts=w_ts, half=half:
                           nc.tensor.matmul(
                               psq, w_ts[c], xt[c][:, half * 512:(half + 1) * 512],
                               start=(c == 0), stop=(c == DCH - 1)))
            boff = p if which == "q" else NPAIR + p

            def evac(psq=psq, dst=dst, half=half, boff=boff):
                with tc.high_priority(offset=400):
                    nc.vector.tensor_scalar_add(
                        out=dst[:, half * 512:(half + 1) * 512], in0=psq,
                        scalar1=bqk_t[:, boff:boff + 1])
            return mms, evac

        def v_chunk(q, s):
            psv = ps.tile([128, 260], F32, name="psv", tag="proj", bufs=2)
            mms = []
            for c in range(DCH):
                mms.append(lambda c=c, psv=psv, q=q, s=s: nc.tensor.matmul(
                    psv, xt[c][:, s * 128:(s + 1) * 128], wv_ts[q][c],
                    start=(c == 0), stop=(c == DCH - 1)))
            vt = v_p.tile([128, 260], F16, name="v_sb", bufs=3 * SBLK)
            v_sb[(q, s)] = vt

            def evac(psv=psv, vt=vt, q=q):
                with tc.high_priority(offset=400):
                    nc.vector.tensor_add(out=vt, in0=psv,
                                         in1=bv_t[:, q * 260:(q + 1) * 260])
            return mms, evac

        # ---- background stream with hard deadlines ----
        bg = []

        def push_qk(p):
            for which, half in (("q", 0), ("k", 0), ("q", 1), ("k", 1)):
                bg.append(qk_chunk(p, which, half))

        def push_v(q):
            for s in range(SBLK):
                bg.append(v_chunk(q, s))

        # chunks ordered by deadline (step index by which the chunk's
        # output must exist); q/k halves of pair p are due just before the
        # pair's query halves start, v-quad n chunk s just before ctx(2n,0,s)
        bg_deadlines = []

        def push_qk(p):
            for (which, half), dl in ((("q", 0), 16 * p - 4),
                                      (("k", 0), 16 * p - 2),
                                      (("q", 1), 16 * p + 4),
                                      (("k", 1), 16 * p + 6)):
                bg.append(qk_chunk(p, which, half))
                bg_deadlines.append(dl)

        def push_v(q):
            for s in range(SBLK):
                bg.append(v_chunk(q, s))
                bg_deadlines.append(32 * q + s - 1)

        push_qk(1)
        push_qk(2)
        push_v(1)
        push_qk(3)
        push_qk(4)
        push_v(2)
        push_qk(5)
        order = sorted(range(len(bg)), key=lambda i: bg_deadlines[i])
        bg[:] = [bg[i] for i in order]
        bg_deadlines[:] = sorted(bg_deadlines)
        # cumulative end position of each chunk in matmuls
        bg_pos = []
        acc = 0
        for mms, _ in bg:
            acc += len(mms)
            bg_pos.append(acc)

        bg_state = {"mms": [], "evac": None, "emitted": 0}

        def bg_pop(n):
            for _ in range(n):
                if not bg_state["mms"]:
                    if bg_state["evac"] is not None:
                        bg_state["evac"]()
                        bg_state["evac"] = None
                    if not bg:
                        return
                    bg_state["mms"], bg_state["evac"] = bg.pop(0)
                bg_state["mms"].pop(0)()
                bg_state["emitted"] += 1
            if not bg_state["mms"] and bg_state["evac"] is not None:
                bg_state["evac"]()
                bg_state["evac"] = None

        def bg_flush_to(pos):
            while bg_state["emitted"] < pos and (
                    bg or bg_state["mms"] or bg_state["evac"]):
                bg_pop(6)

        def bg_flush():
            while bg or bg_state["mms"] or bg_state["evac"]:
                bg_pop(6)

        # ---- startup: pair-0 q/k dense; quad-0 v rides inline in pair 0 ----
        for which, half in (("q", 0), ("k", 0), ("q", 1), ("k", 1)):
            mms, evac = qk_chunk(0, which, half)
            for m in mms:
                m()
            evac()
        startup_v = [v_chunk(0, s) for s in range(SBLK)]

        # denominator scratch: rows 1-63 stay 1.0 forever (reciprocal_approx
        # must run at partition base 0; garbage rows would turn into NaN via
        # 0*inf in the selector matmul)
        rpk_bufs = [z_p.tile([65, 512], F32, name="rpk%d" % i, bufs=1)
                    for i in range(3)]
        rinv_bufs = [z_p.tile([65, 512], F32, name="rinv%d" % i, bufs=1)
                     for i in range(3)]
        rinv16_bufs = [z_p.tile([65, 512], F16, name="rinv16%d" % i, bufs=1)
                       for i in range(3)]
        for t in rpk_bufs:
            nc.vector.memset(t, 1.0)

        ctxt = [cx_p.tile([128, S], F16, name="ctxt", bufs=NPAIR)
                for _ in range(NPAIR)]

        # ---- the fused attention stream ----
        steps = [(p, iblk, j) for p in range(NPAIR) for iblk in range(2)
                 for j in range(SBLK)]
        ets = {}
        pcx_cur = {}

        def emit_scores(p, iblk, j):
            qt, kt = qt_tiles[p], kt_tiles[p]
            pst = ps.tile([128, 1024], F32, name="psst", tag="st", bufs=2)
            nc.tensor.matmul(
                pst[:, 0:512], kt[0:64, j * 128:(j + 1) * 128],
                qt[0:64, iblk * 512:(iblk + 1) * 512],
                start=True, stop=True, tile_position=(0, 0))
            nc.tensor.matmul(
                pst[:, 512:1024], kt[64:128, j * 128:(j + 1) * 128],
                qt[64:128, iblk * 512:(iblk + 1) * 512],
                start=True, stop=True, tile_position=(64, 0))
            et = e_p.tile([128, 1024], F16, name="expt", bufs=6)
            ei = nc.scalar.activation(et, pst, AF.Exp, bias=mask_t[:, j:j + 1])
            exp_insts.append(ei)
            ets[(p, iblk, j)] = et

        def emit_ctx(p, iblk, j):
            q, l0 = divmod(2 * p, 4)
            if j == 0:
                pcx_cur[(p, iblk)] = [
                    ps.tile([65, 512], F32, name="pscx", tag="cx", bufs=2)
                    for _ in range(2)]
            pcx = pcx_cur[(p, iblk)]
            etc = ets.pop((p, iblk, j))
            for idx in range(2):
                vsl = v_sb[(q, j)][:, (l0 + idx) * 65:(l0 + idx + 1) * 65]
                nc.tensor.matmul(pcx[idx], vsl,
                                 etc[:, idx * 512:(idx + 1) * 512],
                                 start=(j == 0), stop=(j == SBLK - 1))

        def emit_finish(p, iblk):
            # evacuate ctx + denominators, then normalize off-path
            ct = ctxt[p]
            pcx = pcx_cur.pop((p, iblk))
            rpk = rpk_bufs[(2 * p + iblk) % 3]
            with tc.high_priority(offset=500):
                for idx in range(2):
                    nc.vector.tensor_copy(
                        out=ct[idx * 64:(idx + 1) * 64,
                               iblk * 512:(iblk + 1) * 512],
                        in_=pcx[idx][0:64, :])
                    nc.vector.tensor_copy(out=rpk[idx * 64:idx * 64 + 1, :],
                                          in_=pcx[idx][64:65, :])
            rinv = rinv_bufs[(2 * p + iblk) % 3]
            rinv16 = rinv16_bufs[(2 * p + iblk) % 3]
            with tc.high_priority(offset=-300):
                with nc.allow_low_precision(reason="softmax denom approx"):
                    nc.vector.reciprocal_approx_fast(out=rinv, in_=rpk)
                nc.vector.tensor_copy(out=rinv16, in_=rinv)
                pbc = ps.tile([128, 512], F32, name="psbc", tag="proj", bufs=2)
                nc.tensor.matmul(pbc, sel_t, rinv16[:, :], start=True,
                                 stop=True)
                for idx in range(2):
                    csl = ct[idx * 64:(idx + 1) * 64,
                             iblk * 512:(iblk + 1) * 512]
                    nc.vector.tensor_mul(out=csl, in0=csl,
                                         in1=pbc[idx * 64:(idx + 1) * 64, :])

        if not trivial_ln:
            gamma_t = const.tile([128, D], F32)
            nc.sync.dma_start(out=gamma_t,
                              in_=gamma_d[0:1, :].to_broadcast([128, D]))
            beta_t = const.tile([128, D], F32)
            nc.sync.dma_start(out=beta_t,
                              in_=beta_d[0:1, :].to_broadcast([128, D]))

        outproj_fin = []

        def emit_outproj(s, chunked=False):
            # chunked: run through the (free) proj-tag slots so the block can
            # overlap pair 5's second query half, whose st slots are busy
            if chunked:
                pso_a = ps.tile([128, 512], F32, name="pso_a", tag="proj",
                                bufs=2)
                pso_b = ps.tile([128, 256], F32, name="pso_b", tag="proj",
                                bufs=2)
                chunks = ((pso_a, 0, 512), (pso_b, 512, 768))
            else:
                pso = ps.tile([128, D], F32, name="pso", tag="st", bufs=2,
                              padded_shape=[128, 1024])
                chunks = ((pso, 0, 512), (pso, 512, 768))
            for buf, d0, d1 in chunks:
                dst = buf if chunked else buf[:, d0:d1]
                if chunked and d0 == 512:
                    dst = buf
                for p in range(NPAIR):
                    nc.tensor.matmul(
                        dst,
                        ctxt[p][:, s * 128:(s + 1) * 128],
                        wo_t[p][:, d0:d1],
                        start=(p == 0), stop=False)
                nc.tensor.matmul(dst, onesr_t, bor_t[:, d0:d1],
                                 start=False, stop=True)
            stats = z_p.tile([128, 3, 6], F32, name="stats", bufs=4)
            if chunked:
                srcs = (pso_a[:, 0:256], pso_a[:, 256:512], pso_b[:, :])
            else:
                srcs = tuple(pso[:, g * 256:(g + 1) * 256] for g in range(3))
            for g in range(3):
                nc.vector.bn_stats(out=stats[:, g, :], in_=srcs[g])
            mv = z_p.tile([128, 2], F32, name="mv", bufs=4)
            nc.vector.bn_aggr(out=mv, in_=stats)
            # evacuate pso now (ACT is idle here) so the psum slots free for
            # the next block's matmuls; the LN finish works from SBUF
            y_sb = z_p.tile([128, D], F32, name="y_sb", bufs=4)
            if chunked:
                nc.scalar.copy(out=y_sb[:, 0:512], in_=pso_a)
                nc.scalar.copy(out=y_sb[:, 512:768], in_=pso_b)
            else:
                nc.scalar.copy(out=y_sb, in_=pso)

            def fin(s=s, mv=mv, y_sb=y_sb):
                # ACT Sqrt deferred past the last exp so the table set is
                # switched exactly once
                stdv = z_p.tile([128, 1], F32, name="stdv", bufs=3)
                nc.scalar.activation(stdv, mv[:, 1:2], AF.Sqrt, bias=eps_t)
                rstd = z_p.tile([128, 1], F32, name="rstd", bufs=3)
                nc.vector.reciprocal(out=rstd, in_=stdv)
                nmr = z_p.tile([128, 1], F32, name="nmr", bufs=3)
                nc.vector.tensor_scalar(out=nmr, in0=mv[:, 0:1], scalar1=rstd,
                                        scalar2=-1.0,
                                        op0=mybir.AluOpType.mult,
                                        op1=mybir.AluOpType.mult)
                z = z_p.tile([128, D], F32, name="z_sb", bufs=3)
                nc.scalar.activation(z, y_sb, AF.Identity, bias=nmr,
                                     scale=rstd)
                if not trivial_ln:
                    nc.vector.tensor_mul(out=z, in0=z, in1=gamma_t)
                    nc.vector.tensor_add(out=z, in0=z, in1=beta_t)
                nc.sync.dma_start(out=out_d[s * 128:(s + 1) * 128, :], in_=z)
            outproj_fin.append(fin)

        bg_done0 = len(bg)

        def bg_required(sti):
            # matmuls that must be emitted by now to satisfy every upcoming
            # chunk deadline at a constant rate
            emitted = bg_state["emitted"]
            req = 0
            base = bg_done0 - len(bg)
            first = base - 1 if (bg_state["mms"] or bg_state["evac"]) else base
            for ci in range(max(0, first), bg_done0):
                pos, dl = bg_pos[ci], bg_deadlines[ci]
                if pos <= emitted:
                    continue
                if dl <= sti:
                    req = max(req, pos - emitted)
                else:
                    req = max(req, -(-(pos - emitted) // (dl - sti)))
            return req

        prev = None
        for sti in range(len(steps) + 1):
            cur = steps[sti] if sti < len(steps) else None
            if cur is not None:
                p, iblk, j = cur
                emit_scores(p, iblk, j)
                if p == 0 and iblk == 0:
                    mms, evac = startup_v[j]
                    for m in mms:
                        m()
                    evac()
            # background projection matmuls keep the PE dense while ACT
            # works through the exp queue; pace to the chunk deadlines
            bg_pop(max(2, bg_required(sti)))
            if prev is not None:
                pp, piblk, pj = prev
                emit_ctx(pp, piblk, pj)
                if pj == SBLK - 1:
                    emit_finish(pp, piblk)
                    if pp == NPAIR - 1 and piblk == 0:
                        # query half 0 fully normalized: overlap the first
                        # out-projection blocks with pair 5's half 1
                        for s in range(3):
                            emit_outproj(s, chunked=True)
            prev = cur

        bg_flush()
        for s in range(3, SBLK):
            emit_outproj(s)
        for fin in outproj_fin:
            fin()

        # defer non-critical weight DMAs behind early exps
        for key, gate in dma_gate.items():
            if gate < len(exp_insts):
                add_dep_helper(dma_insts[key].ins, exp_insts[gate].ins,
                               sync=True,
                               reason="defer weight DMA past startup")

    nc.compile()
    return nc


def _host_inputs(inputs):
    x = np.asarray(inputs["input_tensor"], np.float32)
    mask = np.asarray(inputs["attention_mask"])
    Wq = np.asarray(inputs["Wq"], np.float32)
    bq = np.asarray(inputs["bq"], np.float32)
    Wk = np.asarray(inputs["Wk"], np.float32)
    bk = np.asarray(inputs["bk"], np.float32)
    Wv = np.asarray(inputs["Wv"], np.float32)
    bv = np.asarray(inputs["bv"], np.float32)
    Wo = np.asarray(inputs["Wo"], np.float32)
    bo = np.asarray(inputs["bo"], np.float32)
    gamma = np.asarray(inputs["gamma"], np.float32)
    beta = np.asarray(inputs["beta"], np.float32)

    scale = np.float32(1.0 / np.sqrt(DH))
    wq_flat = np.ascontiguousarray(
        (Wq * scale).transpose(1, 0, 2).reshape(D, D))
    wk_flat = np.ascontiguousarray(Wk.transpose(1, 0, 2).reshape(D, D))
    bq_s = (bq * scale).reshape(D)
    bk_s = bk.reshape(D)

    wv_aug = np.zeros((D, NQUAD * 260), np.float32)
    bv_aug = np.zeros((1, NQUAD * 260), np.float32)
    for h in range(H):
        q, l = divmod(h, 4)
        base = q * 260 + l * 65
        wv_aug[:, base:base + 64] = Wv[h]
        bv_aug[0, base:base + 64] = bv[h]
        bv_aug[0, base + 64] = 1.0

    bqk = np.zeros((128, 2 * NPAIR), np.float32)
    for p in range(NPAIR):
        bqk[:, p] = bq_s[p * 128:(p + 1) * 128]
        bqk[:, NPAIR + p] = bk_s[p * 128:(p + 1) * 128]

    def sbuf_layout(w, width):
        n = w.shape[1] // width
        return np.ascontiguousarray(
            w.reshape(DCH, 128, n, width).transpose(2, 1, 0, 3).reshape(
                n, 128, DCH * width).astype(np.float16))

    shared = {
        "wq": sbuf_layout(wq_flat, 128), "wk": sbuf_layout(wk_flat, 128),
        "wv": sbuf_layout(wv_aug, 260),
        "wo": sbuf_layout(np.ascontiguousarray(Wo), D)[0],
        "bqk": bqk, "bv": bv_aug,
        "gamma": gamma.reshape(1, D), "beta": beta.reshape(1, D),
        "sel": _sel_matrix(),
        "onesr": np.ones((1, 128), np.float32),
        "bor": bo.reshape(1, D).copy(),
    }
    in_maps = []
    for b in range(B):
        mb = np.where(mask[b], 0.0, NEG_MASK).astype(np.float32)
        in_maps.append({
            **shared,
            "xt": np.ascontiguousarray(
                x[b].T.reshape(DCH, 128, S).transpose(1, 0, 2).reshape(
                    128, DCH * S).astype(np.float16)),
            "maskb": np.ascontiguousarray(mb.reshape(SBLK, 128).T),
        })
    return in_maps


def _sel_matrix():
    sel = np.zeros((65, 128), np.float16)
    sel[0, 0:64] = 1.0
    sel[64, 64:128] = 1.0
    return sel


def _trivial_ln(inputs):
    gamma = np.asarray(inputs["gamma"], np.float32)
    beta = np.asarray(inputs["beta"], np.float32)
    return bool(np.all(gamma == 1.0) and np.all(beta == 0.0))


def _get_program(trivial=True):
    if trivial not in _PROGRAMS:
        _PROGRAMS[trivial] = _build_program(trivial)
    return _PROGRAMS[trivial]


def kernel(**inputs):
    from concourse.bass_utils import run_bass_kernel_spmd

    nc = _get_program(_trivial_ln(inputs))
    in_maps = _host_inputs(inputs)
    res = run_bass_kernel_spmd(nc, in_maps, list(range(B)))
    return np.stack([res.results[b]["out"] for b in range(B)], axis=0)


if __name__ == "__main__":
    rng = np.random.default_rng(0)
    demo = {
        "input_tensor": rng.standard_normal((B, S, D)).astype(np.float32),
        "attention_mask": np.ones((B, S), bool),
        "Wq": rng.standard_normal((H, D, DH)).astype(np.float32) * 0.03,
        "bq": rng.standard_normal((H, DH)).astype(np.float32) * 0.03,
        "Wk": rng.standard_normal((H, D, DH)).astype(np.float32) * 0.03,
        "bk": rng.standard_normal((H, DH)).astype(np.float32) * 0.03,
        "Wv": rng.standard_normal((H, D, DH)).astype(np.float32) * 0.03,
        "bv": rng.standard_normal((H, DH)).astype(np.float32) * 0.03,
        "Wo": rng.standard_normal((D, D)).astype(np.float32) * 0.03,
        "bo": rng.standard_normal((D,)).astype(np.float32) * 0.03,
        "gamma": np.ones((D,), np.float32),
        "beta": np.zeros((D,), np.float32),
    }
    out = kernel(**demo)
    print("kernel ran, out shape", out.shape, "finite:", np.isfinite(out).all())
